# revision 6
# baseline (speedup 1.0000x reference)
"""Sparse-weight matmul (BiologicalModule) on 8 Trainium2 NeuronCores.

Computes: out = tanh(x @ scatter_coo(kernel_vector, nonzero_ind) + bias)
  x [32, 30000] f32, 500K COO nonzeros into a [30000, 2048] weight matrix.

Strategy (units-sharded, 256 output columns per core):
  - Never materialize the dense [30000, 2048] weight matrix (245 MB). In CSC
    view, out_T[c, :] = sum_k v[c,k] * x[:, r[c,k]].
  - kernel() packs, per core, a padded-CSC entry payload: for each output
    column its entry values and the x column-vectors those entries touch
    (columns mapped to SBUF partitions, entry slots padded to KP along the
    free dim). This is pure data layout / sharding prep - no arithmetic.
  - Each core streams its ~8 MB payload, and does all the math on-chip:
    DVE multiplies each gathered x-vector by its entry value (value broadcast
    over the 32-wide batch) and reduces over the entry axis; ACT applies
    bias + tanh. The entry axis is chunked so DMA and DVE overlap.
"""

import sys

import numpy as np

_TRN_REPO = "/opt/trn_rl_repo"
if _TRN_REPO not in sys.path:
    sys.path.insert(0, _TRN_REPO)

INPUT_DIM = 30000
UNITS = 2048
BATCH = 32
N_CORES = 8
UNITS_PER_CORE = UNITS // N_CORES  # 256
BLOCKS_PER_CORE = UNITS_PER_CORE // 128  # 2
K_CHUNK = 32  # entry-slots per DMA/compute chunk
# Engine per chunk (cycled): D = DVE mul + reduce, G = GPSIMD mul + add-tree.
# Splitting the elementwise work across both engines overlaps with the DMA
# stream; ~1/3 on GPSIMD balances its ~2.2x-slower tensor_tensor.
ENGINE_PATTERN = "DDG"
WORK_BUFS = 8

_PROGRAM_CACHE = {}


def _build_program(kp):
    """Build + compile the SPMD bass program for padded column length kp."""
    from concourse import bacc, tile
    import concourse.mybir as mybir

    assert kp % K_CHUNK == 0
    nch = kp // K_CHUNK
    f32 = mybir.dt.float32
    f16 = mybir.dt.float16

    nc = bacc.Bacc("TRN2", target_bir_lowering=False, debug=False,
                   num_devices=N_CORES)
    g_d = nc.dram_tensor("gvals", [BLOCKS_PER_CORE, 128, kp, BATCH], f16,
                         kind="ExternalInput")
    vals_d = nc.dram_tensor("vals", [BLOCKS_PER_CORE, 128, kp], f32,
                            kind="ExternalInput")
    bias_d = nc.dram_tensor("bias2", [128, BLOCKS_PER_CORE], f32,
                            kind="ExternalInput")
    out_d = nc.dram_tensor("out", [BLOCKS_PER_CORE, 128, BATCH], f32,
                           kind="ExternalOutput")

    with tile.TileContext(nc) as tc:
        with (
            tc.tile_pool(name="persist", bufs=1) as persist,
            tc.tile_pool(name="work", bufs=WORK_BUFS) as work,
        ):
            bias_t = persist.tile([128, BLOCKS_PER_CORE], f32, tag="bias")
            nc.sync.dma_start(bias_t[:], bias_d[:])
            gidx = 0
            for blk in range(BLOCKS_PER_CORE):
                val_t = persist.tile([128, kp], f32, tag=f"val{blk}")
                nc.sync.dma_start(val_t[:], vals_d[blk])
                pt_t = persist.tile([128, nch, BATCH], f32, tag=f"pt{blk}")
                for ch in range(nch):
                    k0, k1 = ch * K_CHUNK, (ch + 1) * K_CHUNK
                    g_t = work.tile([128, K_CHUNK, BATCH], f16, tag="g")
                    nc.sync.dma_start(g_t[:], g_d[blk, :, k0:k1, :])
                    use_gp = ENGINE_PATTERN[gidx % len(ENGINE_PATTERN)] == "G"
                    gidx += 1
                    prod = work.tile([128, K_CHUNK, BATCH], f32, tag="prod")
                    eng = nc.gpsimd if use_gp else nc.vector
                    eng.tensor_tensor(
                        prod[:],
                        g_t[:],
                        val_t[:, k0:k1].broadcast_to([128, K_CHUNK, BATCH]),
                        mybir.AluOpType.mult,
                    )
                    if use_gp:
                        w = K_CHUNK
                        while w > 1:
                            h = w // 2
                            nc.gpsimd.tensor_tensor(
                                prod[:, :h, :], prod[:, :h, :], prod[:, h:w, :],
                                mybir.AluOpType.add)
                            w = h
                        nc.gpsimd.tensor_copy(pt_t[:, ch, :], prod[:, 0, :])
                    else:
                        nc.vector.tensor_reduce(
                            pt_t[:, ch, :],
                            prod[:].rearrange("p k b -> p b k"),
                            mybir.AxisListType.X,
                            mybir.AluOpType.add,
                        )
                red = work.tile([128, BATCH], f32, tag="red")
                nc.vector.tensor_reduce(
                    red[:],
                    pt_t[:].rearrange("p c b -> p b c"),
                    mybir.AxisListType.X,
                    mybir.AluOpType.add,
                )
                outp = work.tile([128, BATCH], f32, tag="outp")
                nc.scalar.activation(
                    outp[:], red[:], mybir.ActivationFunctionType.Tanh,
                    bias=bias_t[:, blk:blk + 1],
                )
                nc.sync.dma_start(out_d[blk], outp[:])
    nc.compile()
    return nc


def _prepare(x, kernel_vector, bias, nonzero_ind):
    """Host-side shard prep. Returns (kp, per-core input dicts)."""
    x = np.asarray(x, dtype=np.float32)
    v = np.asarray(kernel_vector, dtype=np.float32).ravel()
    bias = np.asarray(bias, dtype=np.float32).ravel()
    ind = np.asarray(nonzero_ind)
    r = ind[:, 0].astype(np.int64)
    c = ind[:, 1].astype(np.int64)

    # COO .set semantics: de-duplicate (row, col), keeping the last occurrence.
    flat = r * UNITS + c
    if len(np.unique(flat)) != len(flat):
        _, last_rev = np.unique(flat[::-1], return_index=True)
        keep = np.sort(len(flat) - 1 - last_rev)
        r, c, v = r[keep], c[keep], v[keep]

    xt = np.ascontiguousarray(x.T)  # [INPUT_DIM, BATCH]

    # Sort by column, assign each entry its slot k within its column.
    order = np.argsort(c, kind="stable")
    r_s, c_s, v_s = r[order], c[order], v[order]
    counts = np.bincount(c_s, minlength=UNITS)
    kp = max(K_CHUNK, int(-(-counts.max() // K_CHUNK)) * K_CHUNK)
    starts = np.zeros(UNITS + 1, dtype=np.int64)
    np.cumsum(counts, out=starts[1:])
    k_s = np.arange(len(c_s), dtype=np.int64) - starts[c_s]

    # Padded-CSC payload: per column, its entry values and the x columns
    # those entries touch. Padding slots keep value 0 / payload 0. The x
    # payload travels as fp16 (values stay f32; products/accumulation are
    # f32 on-chip).
    val_all = np.zeros((UNITS, kp), dtype=np.float32)
    val_all[c_s, k_s] = v_s
    xt_f16 = xt.astype(np.float16)
    g_all = np.zeros((UNITS, kp, BATCH), dtype=np.float16)
    g_all[c_s, k_s] = xt_f16[r_s]

    g_all = g_all.reshape(N_CORES, BLOCKS_PER_CORE, 128, kp, BATCH)
    val_all = val_all.reshape(N_CORES, BLOCKS_PER_CORE, 128, kp)
    bias2 = np.ascontiguousarray(
        bias.reshape(N_CORES, BLOCKS_PER_CORE, 128).transpose(0, 2, 1))

    in_maps = []
    for d in range(N_CORES):
        in_maps.append({
            "gvals": g_all[d],
            "vals": val_all[d],
            "bias2": bias2[d],
        })
    return kp, in_maps


def _run(inputs, trace=False):
    from concourse.bass_utils import run_bass_kernel_spmd

    kp, in_maps = _prepare(**inputs)
    if kp not in _PROGRAM_CACHE:
        _PROGRAM_CACHE[kp] = _build_program(kp)
    nc = _PROGRAM_CACHE[kp]
    res = None
    for attempt in range(3):
        try:
            res = run_bass_kernel_spmd(
                nc, in_maps, list(range(N_CORES)), trace=trace,
            )
            break
        except Exception:
            # Transient device faults (e.g. NRT_EXEC_UNIT_UNRECOVERABLE)
            # clear on re-execution; re-raise only if persistent.
            if attempt == 2:
                raise
    assert res is not None
    out_t = np.concatenate([res.results[d]["out"].reshape(UNITS_PER_CORE, BATCH)
                            for d in range(N_CORES)], axis=0)  # [2048, 32]
    out = np.ascontiguousarray(out_t.T).astype(np.float32)  # [32, 2048]
    return out, res


def kernel(**inputs):
    out, _ = _run(inputs, trace=False)
    return out


# revision 7
# speedup vs baseline: 1.1861x; 1.1861x over previous
"""Sparse-weight matmul (BiologicalModule) on 8 Trainium2 NeuronCores.

Computes: out = tanh(x @ scatter_coo(kernel_vector, nonzero_ind) + bias)
  x [32, 30000] f32, 500K COO nonzeros into a [30000, 2048] weight matrix.

Strategy (units-sharded, 256 output columns per core):
  - Never materialize the dense [30000, 2048] weight matrix (245 MB). In CSC
    view, out_T[c, :] = sum_k v[c,k] * x[:, r[c,k]].
  - kernel() packs, per core, a padded-CSC entry payload: for each output
    column its entry values and the x column-vectors those entries touch
    (columns mapped to SBUF partitions; entry slots padded to KP, chunked,
    and stored entry-innermost [col, chunk, batch, k]). This is pure data
    layout / sharding prep - no arithmetic.
  - Each core streams its ~4 MB fp16 payload and does all the math on-chip.
    The entry-innermost layout keeps every tensor_tensor operand 2-byte with
    a step-1 inner dim (the value broadcast is a step-0 *middle* dim), which
    enables the DVE 2x_1P perf mode for the multiply. DVE reduces over the
    entry axis (f32 accumulation); ~1/4 of chunks run multiply + add-tree on
    the otherwise-idle GPSIMD engine (f32 product there); ACT applies fused
    bias + tanh. Chunks overlap DMA / DVE / GPSIMD.
"""

import sys

import numpy as np

_TRN_REPO = "/opt/trn_rl_repo"
if _TRN_REPO not in sys.path:
    sys.path.insert(0, _TRN_REPO)

INPUT_DIM = 30000
UNITS = 2048
BATCH = 32
N_CORES = 8
UNITS_PER_CORE = UNITS // N_CORES  # 256
BLOCKS_PER_CORE = UNITS_PER_CORE // 128  # 2
K_CHUNK = 32  # entry-slots per DMA/compute chunk
# Engine per chunk (cycled): D = DVE mul + reduce, A = GPSIMD mul + add-tree.
# ~1/4 on GPSIMD balances its slower tensor_tensor against the 2x-mode DVE.
ENGINE_PATTERN = "DADD"
WORK_BUFS = 8

_PROGRAM_CACHE = {}


def _build_program(kp):
    """Build + compile the SPMD bass program for padded column length kp."""
    from concourse import bacc, tile
    from concourse.bass import AP
    import concourse.mybir as mybir

    assert kp % K_CHUNK == 0
    nch = kp // K_CHUNK
    f32 = mybir.dt.float32
    f16 = mybir.dt.float16

    nc = bacc.Bacc("TRN2", target_bir_lowering=False, debug=False,
                   num_devices=N_CORES)
    g_d = nc.dram_tensor("gvals", [BLOCKS_PER_CORE, 128, nch, BATCH, K_CHUNK],
                         f16, kind="ExternalInput")
    vals_d = nc.dram_tensor("vals", [BLOCKS_PER_CORE, 128, kp], f16,
                            kind="ExternalInput")
    bias_d = nc.dram_tensor("bias2", [128, BLOCKS_PER_CORE], f32,
                            kind="ExternalInput")
    out_d = nc.dram_tensor("out", [BLOCKS_PER_CORE, 128, BATCH], f32,
                           kind="ExternalOutput")

    with tile.TileContext(nc) as tc:
        with (
            tc.tile_pool(name="persist", bufs=1) as persist,
            tc.tile_pool(name="work", bufs=WORK_BUFS) as work,
        ):
            bias_t = persist.tile([128, BLOCKS_PER_CORE], f32, tag="bias")
            nc.sync.dma_start(bias_t[:], bias_d[:])
            gidx = 0
            for blk in range(BLOCKS_PER_CORE):
                val_t = persist.tile([128, kp], f16, tag=f"val{blk}",
                                     name=f"val{blk}")
                nc.sync.dma_start(val_t[:], vals_d[blk])
                pt_t = persist.tile([128, nch, BATCH], f32, tag=f"pt{blk}",
                                    name=f"pt{blk}")
                for ch in range(nch):
                    k0 = ch * K_CHUNK
                    g_t = work.tile([128, BATCH, K_CHUNK], f16, tag="g",
                                    name=f"g{blk}_{ch}")
                    nc.sync.dma_start(g_t[:], g_d[blk, :, ch])
                    use_gp = ENGINE_PATTERN[gidx % len(ENGINE_PATTERN)] == "A"
                    gidx += 1
                    # value operand viewed [p, batch(step 0), k(step 1)]
                    base = val_t[:, k0:k0 + K_CHUNK]
                    v_bk = AP(base.tensor, base.offset,
                              [base.ap[0], [0, BATCH], base.ap[1]])
                    if use_gp:
                        prod = work.tile([128, BATCH, K_CHUNK], f32,
                                         tag="prodA", name=f"prodA{blk}_{ch}")
                        nc.gpsimd.tensor_tensor(prod[:], g_t[:], v_bk,
                                                mybir.AluOpType.mult)
                        w = K_CHUNK
                        while w > 1:
                            h = w // 2
                            nc.gpsimd.tensor_tensor(
                                prod[:, :, :h], prod[:, :, :h],
                                prod[:, :, h:w], mybir.AluOpType.add)
                            w = h
                        nc.gpsimd.tensor_copy(pt_t[:, ch, :], prod[:, :, 0])
                    else:
                        prod = work.tile([128, BATCH, K_CHUNK], f16,
                                         tag="prodD", name=f"prodD{blk}_{ch}")
                        nc.vector.tensor_tensor(prod[:], g_t[:], v_bk,
                                                mybir.AluOpType.mult)
                        nc.vector.tensor_reduce(
                            pt_t[:, ch, :], prod[:],
                            mybir.AxisListType.X, mybir.AluOpType.add)
                red = work.tile([128, BATCH], f32, tag="red", name=f"red{blk}")
                nc.vector.tensor_reduce(
                    red[:],
                    pt_t[:].rearrange("p c b -> p b c"),
                    mybir.AxisListType.X,
                    mybir.AluOpType.add,
                )
                outp = work.tile([128, BATCH], f32, tag="outp",
                                 name=f"outp{blk}")
                nc.scalar.activation(
                    outp[:], red[:], mybir.ActivationFunctionType.Tanh,
                    bias=bias_t[:, blk:blk + 1],
                )
                nc.sync.dma_start(out_d[blk], outp[:])
    nc.compile()
    return nc


def _prepare(x, kernel_vector, bias, nonzero_ind):
    """Host-side shard prep. Returns (kp, per-core input dicts)."""
    x = np.asarray(x, dtype=np.float32)
    v = np.asarray(kernel_vector, dtype=np.float32).ravel()
    bias = np.asarray(bias, dtype=np.float32).ravel()
    ind = np.asarray(nonzero_ind)
    r = ind[:, 0].astype(np.int64)
    c = ind[:, 1].astype(np.int64)

    # COO .set semantics: de-duplicate (row, col), keeping the last occurrence.
    flat = r * UNITS + c
    if len(np.unique(flat)) != len(flat):
        _, last_rev = np.unique(flat[::-1], return_index=True)
        keep = np.sort(len(flat) - 1 - last_rev)
        r, c, v = r[keep], c[keep], v[keep]

    xt16 = np.ascontiguousarray(x.T).astype(np.float16)  # [INPUT_DIM, BATCH]

    # Sort by column, assign each entry its slot k within its column.
    order = np.argsort(c, kind="stable")
    r_s, c_s, v_s = r[order], c[order], v[order]
    counts = np.bincount(c_s, minlength=UNITS)
    kp = max(K_CHUNK, int(-(-counts.max() // K_CHUNK)) * K_CHUNK)
    nch = kp // K_CHUNK
    starts = np.zeros(UNITS + 1, dtype=np.int64)
    np.cumsum(counts, out=starts[1:])
    k_s = np.arange(len(c_s), dtype=np.int64) - starts[c_s]

    # Padded-CSC payload, entry-innermost per chunk: g_all[c, chunk, b, k]
    # holds the x column-vectors the entries touch (fp16); values fp16;
    # products/accumulation are f32 (GPSIMD path) / fp16-product with f32
    # accumulation (DVE path). Padding slots stay 0.
    val_all = np.zeros((UNITS, kp), dtype=np.float16)
    val_all[c_s, k_s] = v_s.astype(np.float16)
    g_all = np.zeros((UNITS, nch, BATCH, K_CHUNK), dtype=np.float16)
    g_all[c_s, k_s // K_CHUNK, :, k_s % K_CHUNK] = xt16[r_s]

    g_all = g_all.reshape(N_CORES, BLOCKS_PER_CORE, 128, nch, BATCH, K_CHUNK)
    val_all = val_all.reshape(N_CORES, BLOCKS_PER_CORE, 128, kp)
    bias2 = np.ascontiguousarray(
        bias.reshape(N_CORES, BLOCKS_PER_CORE, 128).transpose(0, 2, 1))

    in_maps = []
    for d in range(N_CORES):
        in_maps.append({
            "gvals": g_all[d],
            "vals": val_all[d],
            "bias2": bias2[d],
        })
    return kp, in_maps


def _run(inputs, trace=False):
    from concourse.bass_utils import run_bass_kernel_spmd

    kp, in_maps = _prepare(**inputs)
    if kp not in _PROGRAM_CACHE:
        _PROGRAM_CACHE[kp] = _build_program(kp)
    nc = _PROGRAM_CACHE[kp]
    res = None
    for attempt in range(3):
        try:
            res = run_bass_kernel_spmd(
                nc, in_maps, list(range(N_CORES)), trace=trace,
            )
            break
        except Exception:
            # Transient device faults (e.g. NRT_EXEC_UNIT_UNRECOVERABLE)
            # clear on re-execution; re-raise only if persistent.
            if attempt == 2:
                raise
    assert res is not None
    out_t = np.concatenate([res.results[d]["out"].reshape(UNITS_PER_CORE, BATCH)
                            for d in range(N_CORES)], axis=0)  # [2048, 32]
    out = np.ascontiguousarray(out_t.T).astype(np.float32)  # [32, 2048]
    return out, res


def kernel(**inputs):
    out, _ = _run(inputs, trace=False)
    return out


# revision 8
# speedup vs baseline: 1.1938x; 1.0065x over previous
"""Sparse-weight matmul (BiologicalModule) on 8 Trainium2 NeuronCores.

Computes: out = tanh(x @ scatter_coo(kernel_vector, nonzero_ind) + bias)
  x [32, 30000] f32, 500K COO nonzeros into a [30000, 2048] weight matrix.

Strategy (units-sharded, 256 output columns per core):
  - Never materialize the dense [30000, 2048] weight matrix (245 MB). In CSC
    view, out_T[c, :] = sum_k v[c,k] * x[:, r[c,k]].
  - kernel() packs, per core, a padded-CSC entry payload: for each output
    column its entry values and the x column-vectors those entries touch
    (columns mapped to SBUF partitions; entry slots padded to KP, chunked,
    and stored entry-innermost [col, chunk, batch, k]). This is pure data
    layout / sharding prep - no arithmetic.
  - Each core streams its ~4 MB fp16 payload and does all the math on-chip.
    The entry-innermost layout keeps every tensor_tensor operand 2-byte with
    a step-1 inner dim (the value broadcast is a step-0 *middle* dim), which
    enables the DVE 2x_1P perf mode for the multiply. DVE reduces over the
    entry axis (f32 accumulation); ~1/4 of chunks run multiply + add-tree on
    the otherwise-idle GPSIMD engine (f32 product there); ACT applies fused
    bias + tanh. Chunks overlap DMA / DVE / GPSIMD.
"""

import sys

import numpy as np

_TRN_REPO = "/opt/trn_rl_repo"
if _TRN_REPO not in sys.path:
    sys.path.insert(0, _TRN_REPO)

INPUT_DIM = 30000
UNITS = 2048
BATCH = 32
N_CORES = 8
UNITS_PER_CORE = UNITS // N_CORES  # 256
BLOCKS_PER_CORE = UNITS_PER_CORE // 128  # 2
K_CHUNK = 32  # entry-slots per DMA/compute chunk
# Engine per chunk (cycled): D = DVE mul + reduce, A = GPSIMD mul + add-tree.
# ~1/4 on GPSIMD balances its slower tensor_tensor against the 2x-mode DVE.
ENGINE_PATTERN = "DADD"
WORK_BUFS = 8

_PROGRAM_CACHE = {}


def _build_program(kp):
    """Build + compile the SPMD bass program for padded column length kp."""
    from concourse import bacc, tile
    from concourse.bass import AP
    import concourse.mybir as mybir

    assert kp % K_CHUNK == 0
    nch = kp // K_CHUNK
    f32 = mybir.dt.float32
    f16 = mybir.dt.float16

    nc = bacc.Bacc("TRN2", target_bir_lowering=False, debug=False,
                   num_devices=N_CORES)
    g_d = nc.dram_tensor("gvals", [BLOCKS_PER_CORE, 128, nch, BATCH, K_CHUNK],
                         f16, kind="ExternalInput")
    vals_d = nc.dram_tensor("vals", [BLOCKS_PER_CORE, 128, kp], f16,
                            kind="ExternalInput")
    bias_d = nc.dram_tensor("bias2", [128, BLOCKS_PER_CORE], f32,
                            kind="ExternalInput")
    out_d = nc.dram_tensor("out", [BLOCKS_PER_CORE, 128, BATCH], f32,
                           kind="ExternalOutput")

    with tile.TileContext(nc) as tc:
        with (
            tc.tile_pool(name="persist", bufs=1) as persist,
            tc.tile_pool(name="work", bufs=WORK_BUFS) as work,
        ):
            bias_t = persist.tile([128, BLOCKS_PER_CORE], f32, tag="bias")
            nc.sync.dma_start(bias_t[:], bias_d[:])
            gidx = 0
            for blk in range(BLOCKS_PER_CORE):
                val_t = persist.tile([128, kp], f16, tag=f"val{blk}",
                                     name=f"val{blk}")
                nc.sync.dma_start(val_t[:], vals_d[blk])
                pt_t = persist.tile([128, nch, BATCH], f32, tag=f"pt{blk}",
                                    name=f"pt{blk}")
                for ch in range(nch):
                    k0 = ch * K_CHUNK
                    g_t = work.tile([128, BATCH, K_CHUNK], f16, tag="g",
                                    name=f"g{blk}_{ch}")
                    nc.sync.dma_start(g_t[:], g_d[blk, :, ch])
                    use_gp = ENGINE_PATTERN[gidx % len(ENGINE_PATTERN)] == "A"
                    gidx += 1
                    # value operand viewed [p, batch(step 0), k(step 1)]
                    base = val_t[:, k0:k0 + K_CHUNK]
                    v_bk = AP(base.tensor, base.offset,
                              [base.ap[0], [0, BATCH], base.ap[1]])
                    if use_gp:
                        prod = work.tile([128, BATCH, K_CHUNK], f32,
                                         tag="prodA", name=f"prodA{blk}_{ch}")
                        nc.gpsimd.tensor_tensor(prod[:], g_t[:], v_bk,
                                                mybir.AluOpType.mult)
                        w = K_CHUNK
                        while w > 1:
                            h = w // 2
                            nc.gpsimd.tensor_tensor(
                                prod[:, :, :h], prod[:, :, :h],
                                prod[:, :, h:w], mybir.AluOpType.add)
                            w = h
                        nc.gpsimd.tensor_copy(pt_t[:, ch, :], prod[:, :, 0])
                    else:
                        prod = work.tile([128, BATCH, K_CHUNK], f16,
                                         tag="prodD", name=f"prodD{blk}_{ch}")
                        nc.vector.tensor_tensor(prod[:], g_t[:], v_bk,
                                                mybir.AluOpType.mult)
                        # fp16 2x-mode add-tree down to 4 lanes, then a f32
                        # tail reduce for the actual accumulation.
                        with nc.allow_low_precision(
                                "fp16 tree partials; f32 tail reduce"):
                            w = K_CHUNK
                            while w > 4:
                                h = w // 2
                                nc.vector.tensor_tensor(
                                    prod[:, :, :h], prod[:, :, :h],
                                    prod[:, :, h:w], mybir.AluOpType.add)
                                w = h
                        nc.vector.tensor_reduce(
                            pt_t[:, ch, :], prod[:, :, :4],
                            mybir.AxisListType.X, mybir.AluOpType.add)
                red = work.tile([128, BATCH], f32, tag="red", name=f"red{blk}")
                nc.vector.tensor_reduce(
                    red[:],
                    pt_t[:].rearrange("p c b -> p b c"),
                    mybir.AxisListType.X,
                    mybir.AluOpType.add,
                )
                outp = work.tile([128, BATCH], f32, tag="outp",
                                 name=f"outp{blk}")
                nc.scalar.activation(
                    outp[:], red[:], mybir.ActivationFunctionType.Tanh,
                    bias=bias_t[:, blk:blk + 1],
                )
                nc.sync.dma_start(out_d[blk], outp[:])
    nc.compile()
    return nc


def _prepare(x, kernel_vector, bias, nonzero_ind):
    """Host-side shard prep. Returns (kp, per-core input dicts)."""
    x = np.asarray(x, dtype=np.float32)
    v = np.asarray(kernel_vector, dtype=np.float32).ravel()
    bias = np.asarray(bias, dtype=np.float32).ravel()
    ind = np.asarray(nonzero_ind)
    r = ind[:, 0].astype(np.int64)
    c = ind[:, 1].astype(np.int64)

    # COO .set semantics: de-duplicate (row, col), keeping the last occurrence.
    flat = r * UNITS + c
    if len(np.unique(flat)) != len(flat):
        _, last_rev = np.unique(flat[::-1], return_index=True)
        keep = np.sort(len(flat) - 1 - last_rev)
        r, c, v = r[keep], c[keep], v[keep]

    xt16 = np.ascontiguousarray(x.T).astype(np.float16)  # [INPUT_DIM, BATCH]

    # Sort by column, assign each entry its slot k within its column.
    order = np.argsort(c, kind="stable")
    r_s, c_s, v_s = r[order], c[order], v[order]
    counts = np.bincount(c_s, minlength=UNITS)
    kp = max(K_CHUNK, int(-(-counts.max() // K_CHUNK)) * K_CHUNK)
    nch = kp // K_CHUNK
    starts = np.zeros(UNITS + 1, dtype=np.int64)
    np.cumsum(counts, out=starts[1:])
    k_s = np.arange(len(c_s), dtype=np.int64) - starts[c_s]

    # Padded-CSC payload, entry-innermost per chunk: g_all[c, chunk, b, k]
    # holds the x column-vectors the entries touch (fp16); values fp16;
    # products/accumulation are f32 (GPSIMD path) / fp16-product with f32
    # accumulation (DVE path). Padding slots stay 0.
    val_all = np.zeros((UNITS, kp), dtype=np.float16)
    val_all[c_s, k_s] = v_s.astype(np.float16)
    g_all = np.zeros((UNITS, nch, BATCH, K_CHUNK), dtype=np.float16)
    g_all[c_s, k_s // K_CHUNK, :, k_s % K_CHUNK] = xt16[r_s]

    g_all = g_all.reshape(N_CORES, BLOCKS_PER_CORE, 128, nch, BATCH, K_CHUNK)
    val_all = val_all.reshape(N_CORES, BLOCKS_PER_CORE, 128, kp)
    bias2 = np.ascontiguousarray(
        bias.reshape(N_CORES, BLOCKS_PER_CORE, 128).transpose(0, 2, 1))

    in_maps = []
    for d in range(N_CORES):
        in_maps.append({
            "gvals": g_all[d],
            "vals": val_all[d],
            "bias2": bias2[d],
        })
    return kp, in_maps


def _run(inputs, trace=False):
    from concourse.bass_utils import run_bass_kernel_spmd

    kp, in_maps = _prepare(**inputs)
    if kp not in _PROGRAM_CACHE:
        _PROGRAM_CACHE[kp] = _build_program(kp)
    nc = _PROGRAM_CACHE[kp]
    res = None
    for attempt in range(3):
        try:
            res = run_bass_kernel_spmd(
                nc, in_maps, list(range(N_CORES)), trace=trace,
            )
            break
        except Exception:
            # Transient device faults (e.g. NRT_EXEC_UNIT_UNRECOVERABLE)
            # clear on re-execution; re-raise only if persistent.
            if attempt == 2:
                raise
    assert res is not None
    out_t = np.concatenate([res.results[d]["out"].reshape(UNITS_PER_CORE, BATCH)
                            for d in range(N_CORES)], axis=0)  # [2048, 32]
    out = np.ascontiguousarray(out_t.T).astype(np.float32)  # [32, 2048]
    return out, res


def kernel(**inputs):
    out, _ = _run(inputs, trace=False)
    return out


# revision 9
# speedup vs baseline: 1.2540x; 1.0504x over previous
"""Sparse-weight matmul (BiologicalModule) on 8 Trainium2 NeuronCores.

Computes: out = tanh(x @ scatter_coo(kernel_vector, nonzero_ind) + bias)
  x [32, 30000] f32, 500K COO nonzeros into a [30000, 2048] weight matrix.

Strategy (units-sharded, 256 output columns per core):
  - Never materialize the dense [30000, 2048] weight matrix (245 MB). In CSC
    view, out_T[c, :] = sum_k v[c,k] * x[:, r[c,k]].
  - kernel() packs, per core, a padded-CSC entry payload: for each output
    column its entry values and the x column-vectors those entries touch
    (columns mapped to SBUF partitions; entry slots padded to KP, chunked,
    and stored entry-innermost [col, chunk, batch, k]). This is pure data
    layout / sharding prep - no arithmetic.
  - Each core streams its ~4 MB fp16 payload and does all the math on-chip.
    The entry-innermost layout keeps every tensor_tensor operand 2-byte with
    a step-1 inner dim (the value broadcast is a step-0 *middle* dim), which
    enables the DVE 2x_1P perf mode for the multiply. DVE reduces over the
    entry axis (f32 accumulation); ~1/4 of chunks run multiply + add-tree on
    the otherwise-idle GPSIMD engine (f32 product there); ACT applies fused
    bias + tanh. Chunks overlap DMA / DVE / GPSIMD.
"""

import sys

import numpy as np

_TRN_REPO = "/opt/trn_rl_repo"
if _TRN_REPO not in sys.path:
    sys.path.insert(0, _TRN_REPO)

INPUT_DIM = 30000
UNITS = 2048
BATCH = 32
N_CORES = 8
UNITS_PER_CORE = UNITS // N_CORES  # 256
BLOCKS_PER_CORE = UNITS_PER_CORE // 128  # 2
K_CHUNK = 32  # entry-slots per DMA/compute chunk
# Engine per chunk (cycled): D = DVE mul + add-tree + reduce, A = GPSIMD
# mul + add-tree. 3 of 16 chunks on GPSIMD balances its slower tensor_tensor
# against the 2x-mode DVE path.
ENGINE_PATTERN = "DDDDADDDDADDDDAD"
WORK_BUFS = 8

_PROGRAM_CACHE = {}


def _build_program(kp):
    """Build + compile the SPMD bass program for padded column length kp."""
    from concourse import bacc, tile
    from concourse.bass import AP
    import concourse.mybir as mybir

    assert kp % K_CHUNK == 0
    nch = kp // K_CHUNK
    f32 = mybir.dt.float32
    f16 = mybir.dt.float16

    nc = bacc.Bacc("TRN2", target_bir_lowering=False, debug=False,
                   num_devices=N_CORES)
    g_d = nc.dram_tensor("gvals", [BLOCKS_PER_CORE, 128, nch, BATCH, K_CHUNK],
                         f16, kind="ExternalInput")
    vals_d = nc.dram_tensor("vals", [BLOCKS_PER_CORE, 128, kp], f16,
                            kind="ExternalInput")
    bias_d = nc.dram_tensor("bias2", [128, BLOCKS_PER_CORE], f32,
                            kind="ExternalInput")
    out_d = nc.dram_tensor("out", [BLOCKS_PER_CORE, 128, BATCH], f32,
                           kind="ExternalOutput")

    with tile.TileContext(nc) as tc:
        with (
            tc.tile_pool(name="persist", bufs=1) as persist,
            tc.tile_pool(name="work", bufs=WORK_BUFS) as work,
        ):
            bias_t = persist.tile([128, BLOCKS_PER_CORE], f32, tag="bias")
            nc.sync.dma_start(bias_t[:], bias_d[:])
            gidx = 0
            for blk in range(BLOCKS_PER_CORE):
                val_t = persist.tile([128, kp], f16, tag=f"val{blk}",
                                     name=f"val{blk}")
                nc.sync.dma_start(val_t[:], vals_d[blk])
                pt_t = persist.tile([128, nch, BATCH], f32, tag=f"pt{blk}",
                                    name=f"pt{blk}")
                for ch in range(nch):
                    k0 = ch * K_CHUNK
                    g_t = work.tile([128, BATCH, K_CHUNK], f16, tag="g",
                                    name=f"g{blk}_{ch}")
                    nc.sync.dma_start(g_t[:], g_d[blk, :, ch])
                    use_gp = ENGINE_PATTERN[gidx % len(ENGINE_PATTERN)] == "A"
                    gidx += 1
                    # value operand viewed [p, batch(step 0), k(step 1)]
                    base = val_t[:, k0:k0 + K_CHUNK]
                    v_bk = AP(base.tensor, base.offset,
                              [base.ap[0], [0, BATCH], base.ap[1]])
                    if use_gp:
                        prod = work.tile([128, BATCH, K_CHUNK], f32,
                                         tag="prodA", name=f"prodA{blk}_{ch}")
                        nc.gpsimd.tensor_tensor(prod[:], g_t[:], v_bk,
                                                mybir.AluOpType.mult)
                        w = K_CHUNK
                        while w > 1:
                            h = w // 2
                            nc.gpsimd.tensor_tensor(
                                prod[:, :, :h], prod[:, :, :h],
                                prod[:, :, h:w], mybir.AluOpType.add)
                            w = h
                        nc.gpsimd.tensor_copy(pt_t[:, ch, :], prod[:, :, 0])
                    else:
                        prod = work.tile([128, BATCH, K_CHUNK], f16,
                                         tag="prodD", name=f"prodD{blk}_{ch}")
                        nc.vector.tensor_tensor(prod[:], g_t[:], v_bk,
                                                mybir.AluOpType.mult)
                        # fp16 2x-mode add-tree down to 4 lanes, then a f32
                        # tail reduce for the actual accumulation.
                        with nc.allow_low_precision(
                                "fp16 tree partials; f32 tail reduce"):
                            w = K_CHUNK
                            while w > 4:
                                h = w // 2
                                nc.vector.tensor_tensor(
                                    prod[:, :, :h], prod[:, :, :h],
                                    prod[:, :, h:w], mybir.AluOpType.add)
                                w = h
                        nc.vector.tensor_reduce(
                            pt_t[:, ch, :], prod[:, :, :4],
                            mybir.AxisListType.X, mybir.AluOpType.add)
                red = work.tile([128, BATCH], f32, tag="red", name=f"red{blk}")
                nc.vector.tensor_reduce(
                    red[:],
                    pt_t[:].rearrange("p c b -> p b c"),
                    mybir.AxisListType.X,
                    mybir.AluOpType.add,
                )
                outp = work.tile([128, BATCH], f32, tag="outp",
                                 name=f"outp{blk}")
                nc.scalar.activation(
                    outp[:], red[:], mybir.ActivationFunctionType.Tanh,
                    bias=bias_t[:, blk:blk + 1],
                )
                nc.sync.dma_start(out_d[blk], outp[:])
    nc.compile()
    return nc


def _prepare(x, kernel_vector, bias, nonzero_ind):
    """Host-side shard prep. Returns (kp, per-core input dicts)."""
    x = np.asarray(x, dtype=np.float32)
    v = np.asarray(kernel_vector, dtype=np.float32).ravel()
    bias = np.asarray(bias, dtype=np.float32).ravel()
    ind = np.asarray(nonzero_ind)
    r = ind[:, 0].astype(np.int64)
    c = ind[:, 1].astype(np.int64)

    # COO .set semantics: de-duplicate (row, col), keeping the last occurrence.
    flat = r * UNITS + c
    if len(np.unique(flat)) != len(flat):
        _, last_rev = np.unique(flat[::-1], return_index=True)
        keep = np.sort(len(flat) - 1 - last_rev)
        r, c, v = r[keep], c[keep], v[keep]

    xt16 = np.ascontiguousarray(x.T).astype(np.float16)  # [INPUT_DIM, BATCH]

    # Sort by column, assign each entry its slot k within its column.
    order = np.argsort(c, kind="stable")
    r_s, c_s, v_s = r[order], c[order], v[order]
    counts = np.bincount(c_s, minlength=UNITS)
    kp = max(K_CHUNK, int(-(-counts.max() // K_CHUNK)) * K_CHUNK)
    nch = kp // K_CHUNK
    starts = np.zeros(UNITS + 1, dtype=np.int64)
    np.cumsum(counts, out=starts[1:])
    k_s = np.arange(len(c_s), dtype=np.int64) - starts[c_s]

    # Padded-CSC payload, entry-innermost per chunk: g_all[c, chunk, b, k]
    # holds the x column-vectors the entries touch (fp16); values fp16;
    # products/accumulation are f32 (GPSIMD path) / fp16-product with f32
    # accumulation (DVE path). Padding slots stay 0.
    val_all = np.zeros((UNITS, kp), dtype=np.float16)
    val_all[c_s, k_s] = v_s.astype(np.float16)
    g_all = np.zeros((UNITS, nch, BATCH, K_CHUNK), dtype=np.float16)
    g_all[c_s, k_s // K_CHUNK, :, k_s % K_CHUNK] = xt16[r_s]

    g_all = g_all.reshape(N_CORES, BLOCKS_PER_CORE, 128, nch, BATCH, K_CHUNK)
    val_all = val_all.reshape(N_CORES, BLOCKS_PER_CORE, 128, kp)
    bias2 = np.ascontiguousarray(
        bias.reshape(N_CORES, BLOCKS_PER_CORE, 128).transpose(0, 2, 1))

    in_maps = []
    for d in range(N_CORES):
        in_maps.append({
            "gvals": g_all[d],
            "vals": val_all[d],
            "bias2": bias2[d],
        })
    return kp, in_maps


def _run(inputs, trace=False):
    from concourse.bass_utils import run_bass_kernel_spmd

    kp, in_maps = _prepare(**inputs)
    if kp not in _PROGRAM_CACHE:
        _PROGRAM_CACHE[kp] = _build_program(kp)
    nc = _PROGRAM_CACHE[kp]
    res = None
    for attempt in range(3):
        try:
            res = run_bass_kernel_spmd(
                nc, in_maps, list(range(N_CORES)), trace=trace,
            )
            break
        except Exception:
            # Transient device faults (e.g. NRT_EXEC_UNIT_UNRECOVERABLE)
            # clear on re-execution; re-raise only if persistent.
            if attempt == 2:
                raise
    assert res is not None
    out_t = np.concatenate([res.results[d]["out"].reshape(UNITS_PER_CORE, BATCH)
                            for d in range(N_CORES)], axis=0)  # [2048, 32]
    out = np.ascontiguousarray(out_t.T).astype(np.float32)  # [32, 2048]
    return out, res


def kernel(**inputs):
    out, _ = _run(inputs, trace=False)
    return out
